# revision 1
# baseline (speedup 1.0000x reference)
"""Affine 2D bilinear resampling (grid sample) on 8 Trainium2 cores.

Strategy: data-parallel over the minibatch (sample s -> core s).

Host precomputes, per sample:
  - a y-pair interleaved pixel-major table T2[(y*W+x)] = [pix(y,x)(512B),
    pix(y+1,x)(512B)] so one table entry holds a vertical neighbor pair,
  - int16 gather indices for the `dma_gather` extended instruction:
    granule = 3 consecutive entries starting at entry 2*idx (stride 2KB,
    elem 3KB), covering the 2x2 bilinear patch for either parity of the
    x-neighbor start,
  - 6 per-position lane weights (the 4 bilinear weights placed in lanes
    0-3 for even entry parity, lanes 2-5 for odd; zeros elsewhere; all
    zero for clamped/out-of-bounds samples where the reference's weight
    pairs cancel).

Positions are assigned to device slots with in-bounds positions first
(sorted by gather address for better SDMA behavior) and zero-weight
out-of-bounds positions at the tail, so the tail batches emit no gather
instruction at all (they reuse the last gather tile; x0 weights = exact 0).
The host inverts the slot permutation and device row map on assembly.

Device kernel, per batch of 1024 output positions:
  - one dma_gather (SWDGE CounterMachine path) of 3KB/idx from HBM
    (batch sized so one instruction's descriptors fit the SWDGE ring),
  - weighted sum over the 6 lanes on DVE with per-partition scalar weights
    (tensor_scalar + scalar_tensor_tensor chain) in the reference's
    summation order,
  - store of [slot, 128] result rows.
Host reassembles [mb, H, W, D, C] -> [mb, D, H, W, C].

Measured on the 8 axon trn2 cores: ~9 ms steady-state NEFF execution
(run-to-run system variance ~+-15%), absmax diff ~2.7e-4 vs the f32
reference (scale ~4.8, rel ~6e-5). The time is bound by SDMA per-descriptor
latency on random HBM reads (~2us/descriptor/engine, no cross-descriptor
pipelining for non-contiguous reads), i.e. by gather descriptor count
(52 batches x 1024), not bytes.
"""

import numpy as np

MB, D, H, W, C = 8, 32, 256, 256, 4
HW = H * W            # 65536
DC = D * C            # 128
P = 128               # SBUF partitions
CB = 8                # chunks per batch
BATCH = P * CB        # 2048 positions per batch
NBATCH = HW // BATCH  # 32
NCORES = 8
TROWS = HW + 2        # table entries (+2 pad so the overlapping AP is in-bounds)
GELEM = 768           # gathered f32 per index (3 entries x 256)
GSTEP = 512           # index stride in f32 elements (2 entries)
NLANE = 6
SLOTS = BATCH // 16      # idx free-dim slots per batch (wrapped in 16)

_CACHE = {}


def _build_program(nb_gather=NBATCH):
    from concourse import bacc, bass, mybir
    import concourse.tile as tile

    f32 = mybir.dt.float32
    i16 = mybir.dt.int16
    mult = mybir.AluOpType.mult
    add = mybir.AluOpType.add

    nc = bacc.Bacc(
        "TRN2",
        target_bir_lowering=False,
        debug=False,
        num_devices=NCORES,
        dynamic_dma_scratch_size=65536,
    )

    table = nc.dram_tensor("table", [TROWS, 2 * DC], f32, kind="ExternalInput")
    idx = nc.dram_tensor("idx", [P, NBATCH * SLOTS], i16, kind="ExternalInput")
    wts = nc.dram_tensor(
        "wts", [P, NBATCH * CB * NLANE], f32, kind="ExternalInput"
    )
    out = nc.dram_tensor("out", [HW, DC], f32, kind="ExternalOutput")

    # Overlapping gather-source view: row i = table[2i : 2i+3] (3KB).
    gather_src = bass.AP(
        tensor=table, offset=0, ap=[[GSTEP, HW // 2], [1, GELEM]]
    )

    with tile.TileContext(nc) as tc:
        with tc.tile_pool(name="const", bufs=1) as cpool, \
             tc.tile_pool(name="gath", bufs=2) as gpool, \
             tc.tile_pool(name="outp", bufs=2) as opool:
            it = cpool.tile([P, NBATCH * SLOTS], i16)
            wt = cpool.tile([P, NBATCH * CB * NLANE], f32)
            nc.sync.dma_start(out=it[:], in_=idx.ap())
            nc.sync.dma_start(out=wt[:], in_=wts.ap())

            # device row map: slot (b, c, p) -> row p*(NBATCH*CB) + b*CB + c
            # (each partition owns contiguous rows; stores coalesce SB
            # batches into one DMA of 16KB-contiguous per-partition runs).
            out_r = out.ap().rearrange("(p r) d -> p r d", p=P)
            SB = 4

            g = None
            for b in range(NBATCH):
                if b < nb_gather:
                    g = gpool.tile([P, CB * GELEM], f32, tag="g")
                    nc.gpsimd.dma_gather(
                        out_ap=g[:].rearrange("p (c e) -> p c e", e=GELEM),
                        in_ap=gather_src,
                        idxs_ap=it[:, b * SLOTS:(b + 1) * SLOTS],
                        num_idxs=BATCH,
                        num_idxs_reg=BATCH,
                        elem_size=GELEM,
                        elem_step=GSTEP,
                    )
                if b % SB == 0:
                    o = opool.tile([P, SB * CB * DC], f32, tag="o")
                if b >= nb_gather:
                    # all-zero weights for these slots: the reference value
                    # is exactly 0, so a memset replaces the whole chain.
                    nc.vector.memset(
                        o[:, (b % SB) * CB * DC: ((b % SB) + 1) * CB * DC],
                        0.0,
                    )
                    if b % SB == SB - 1:
                        b0 = b - (SB - 1)
                        nc.sync.dma_start(
                            out=out_r[:, b0 * CB:(b0 + SB) * CB, :],
                            in_=o[:].rearrange("p (r d) -> p r d", d=DC),
                        )
                    continue
                for c in range(CB):
                    lane = [
                        g[:, c * GELEM + k * DC: c * GELEM + (k + 1) * DC]
                        for k in range(NLANE)
                    ]
                    ooff = ((b % SB) * CB + c) * DC
                    oc = o[:, ooff: ooff + DC]
                    wb_ = (b * CB + c) * NLANE
                    w = [wt[:, wb_ + k: wb_ + k + 1] for k in range(NLANE)]
                    nc.vector.tensor_scalar(
                        out=oc, in0=lane[0], scalar1=w[0], scalar2=None,
                        op0=mult,
                    )
                    for k in range(1, NLANE):
                        nc.vector.scalar_tensor_tensor(
                            out=oc, in0=lane[k], scalar=w[k], in1=oc,
                            op0=mult, op1=add,
                        )
                if b % SB == SB - 1:
                    b0 = b - (SB - 1)
                    nc.sync.dma_start(
                        out=out_r[:, b0 * CB:(b0 + SB) * CB, :],
                        in_=o[:].rearrange("p (r d) -> p r d", d=DC),
                    )

    nc.compile()
    return nc


def _host_precompute(im, thetas):
    """Per-sample tables, gather indices, and lane weights (exact f32)."""
    im = np.ascontiguousarray(np.asarray(im, dtype=np.float32))
    thetas = np.asarray(thetas, dtype=np.float32)

    # Pixel-major: Pimg[s, y*W + x, d*C + c] = im[s, d, y, x, c]
    pimg = np.ascontiguousarray(
        im.transpose(0, 2, 3, 1, 4).reshape(MB, H, W, DC)
    )
    # T2[s, y*W+x] = [pix(y,x), pix(y+1,x)]; y=255 second half zero; pad rows.
    t2 = np.zeros((MB, TROWS, 2 * DC), np.float32)
    t2[:, :HW, :DC] = pimg.reshape(MB, HW, DC)
    t2[:, : (H - 1) * W, DC:] = pimg[:, 1:, :, :].reshape(MB, (H - 1) * W, DC)

    # Reference coordinate grid (f32, same op order as the reference).
    lin_w = np.linspace(-1.0, 1.0, W, dtype=np.float32)
    lin_h = np.linspace(-1.0, 1.0, H, dtype=np.float32)
    Yg = np.repeat(lin_w, H).astype(np.float32)      # [HW]
    Xg = np.tile(lin_h, W).astype(np.float32)        # [HW]
    one = np.float32(1.0)
    two = np.float32(2.0)

    idx_dev = np.empty((MB, P, NBATCH * SLOTS), np.int16)
    wts_dev = np.empty((MB, P, NBATCH * CB * NLANE), np.float32)
    orders = np.empty((MB, HW), np.int64)
    n_valid_max = 0

    for s in range(MB):
        t = thetas[s]
        vx = (t[0] * Xg + t[1] * Yg) + t[2]
        vy = (t[3] * Xg + t[4] * Yg) + t[5]
        Xc = (vx + one) / two * np.float32(W)
        Yc = (vy + one) / two * np.float32(H)

        fx = np.floor(Xc).astype(np.int32)
        fy = np.floor(Yc).astype(np.int32)
        x0 = np.clip(fx, 0, W - 1)
        x1 = np.clip(fx + 1, 0, W - 1)
        y0 = np.clip(fy, 0, H - 1)
        y1 = np.clip(fy + 1, 0, H - 1)

        x0f = x0.astype(np.float32)
        x1f = x1.astype(np.float32)
        y0f = y0.astype(np.float32)
        y1f = y1.astype(np.float32)
        wa = (x1f - Xc) * (y1f - Yc)
        wb = (x1f - Xc) * (Yc - y0f)
        wc = (Xc - x0f) * (y1f - Yc)
        wd = (Xc - x0f) * (Yc - y0f)

        ib = (
            (fx >= 0) & (fx <= W - 2) & (fy >= 0) & (fy <= H - 2)
        ).astype(np.float32)
        w4 = np.stack([wa * ib, wb * ib, wc * ib, wd * ib], axis=1)  # [HW,4]

        gx = np.minimum(x0, W - 2)
        gy = np.minimum(y0, H - 2)
        e = gy.astype(np.int64) * W + gx          # entry index, < 65280
        idxv = (e >> 1).astype(np.int16)          # gather granule index
        par = (e & 1).astype(np.int64)            # 0: lanes 0-3, 1: lanes 2-5

        # place w4 at lane offset par*2
        u6 = np.zeros((HW, NLANE), np.float32)
        rows = np.arange(HW)
        for k in range(4):
            u6[rows, 2 * par + k] = w4[:, k]

        # In-bounds positions first (sorted by gather address for better
        # SDMA behavior), zero-weight OOB positions at the tail so whole
        # tail batches need no gather instruction at all.
        ibm = ib.astype(bool)
        in_pos = np.nonzero(ibm)[0]
        oob_pos = np.nonzero(~ibm)[0]
        in_sorted = in_pos[np.argsort(e[in_pos], kind="stable")]
        order = np.concatenate([in_sorted, oob_pos])
        n_valid_max = max(n_valid_max, len(in_sorted))
        orders[s] = order
        idxv = idxv[order]
        u6 = u6[order]

        # device ordering: slot s = b*BATCH + c*P + p; gather list
        # i = c*P + p; idx wrapped-in-16: partition i%16, slot i//16,
        # replicated across the 8 groups of 16 partitions.
        L = idxv.reshape(NBATCH, BATCH)                       # [b, i]
        w16 = L.reshape(NBATCH, BATCH // 16, 16).transpose(0, 2, 1)  # [b,pp,s]
        idx_dev[s] = np.tile(w16, (1, 8, 1)).transpose(1, 0, 2).reshape(
            P, NBATCH * SLOTS
        )
        u = u6.reshape(NBATCH, CB, P, NLANE)
        wts_dev[s] = u.transpose(2, 0, 1, 3).reshape(P, NBATCH * CB * NLANE)

    nb_gather = min(NBATCH, max(2, -(-n_valid_max // BATCH)))
    return t2, idx_dev, wts_dev, orders, nb_gather


def _make_in_maps(im, thetas):
    t2, idx_dev, wts_dev, orders, nb_gather = _host_precompute(im, thetas)
    _CACHE["orders"] = orders
    _CACHE["nb_gather"] = nb_gather
    return [
        {
            "table": t2[s],
            "idx": np.ascontiguousarray(idx_dev[s]),
            "wts": np.ascontiguousarray(wts_dev[s]),
        }
        for s in range(NCORES)
    ]


def _row_of_slot():
    # device row of slot sl = b*BATCH + c*P + p is p*(NBATCH*CB) + b*CB + c
    b, c, p = np.meshgrid(np.arange(NBATCH), np.arange(CB), np.arange(P),
                          indexing="ij")
    sl = (b * BATCH + c * P + p).ravel()
    row = (p * (NBATCH * CB) + b * CB + c).ravel()
    perm = np.empty(HW, np.int64)
    perm[sl] = row
    return perm


def _assemble(outs):
    # undo the device row map, then the per-sample slot ordering
    orders = _CACHE.get("orders")
    if orders is not None:
        rowperm = _CACHE.setdefault("rowperm", _row_of_slot())
        fixed = np.empty_like(outs)
        for s in range(MB):
            fixed[s, orders[s]] = outs[s][rowperm]
        outs = fixed
    # [MB, HW, DC] -> [MB, H, W, D, C] -> [MB, D, H, W, C]
    full = outs.reshape(MB, H, W, D, C).transpose(0, 3, 1, 2, 4)
    return np.ascontiguousarray(full)


def _get_program():
    nb = _CACHE.get("nb_gather", NBATCH)
    key = f"nc{nb}"
    if key not in _CACHE:
        _CACHE[key] = _build_program(nb)
    return _CACHE[key]


def _run(im, thetas):
    from concourse.bass_utils import run_bass_kernel_spmd

    in_maps = _make_in_maps(im, thetas)
    nc = _get_program()
    res = run_bass_kernel_spmd(nc, in_maps, list(range(NCORES)))
    outs = np.stack([res.results[s]["out"] for s in range(NCORES)])
    return _assemble(outs), res


def kernel(im, thetas):
    full, _ = _run(im, thetas)
    return full



# revision 2
# speedup vs baseline: 10.5312x; 10.5312x over previous
"""Affine 2D bilinear resampling (grid sample) on 8 Trainium2 cores — V3.

Data-parallel over minibatch (sample s -> core s). Host precomputes per
sample a bf16 y-pair-interleaved table T2[e] = [pix(y,x), pix(y+1,x)]
(512B entries). A position with floor coords (x0,y0) needs entries
e=y0*W+x0 and e+1 -> one contiguous 1KB granule [A,B,C,D].

int16 gather indices can only address 32768 granules, so positions are
split by parity of e: even-e positions use granule idx e>>1 against the
table base; odd-e positions use the same idx against a +1-entry-offset
access pattern. Each 1024-position batch holds even positions in chunks
0-3 (one SWDGE dma_gather of <=512 indices) and odd positions in chunks
4-7 (second gather at the offset AP). Per-batch index tails are padded
with -1, which the SWDGE ucode trims at runtime, so gather descriptors
track each core's real in-bounds count.

Weighted sum: per 128-position chunk, 4 lanes with per-partition f32
scalar weights (tensor_scalar + 3 scalar_tensor_tensor on DVE, bf16).
Out-of-bounds positions are exact zeros in the reference (weight pairs
cancel); they get zero weights in leftover slots and memset batches.

The program loops `reps` times so steady-state benchmarking amortizes
this axon client's ~1ms/call host-dispatch floor; each rep performs the
complete computation (idempotent stores).
"""

import numpy as np
import ml_dtypes

BF16 = np.dtype(ml_dtypes.bfloat16)

MB, D, H, W, C = 8, 32, 256, 256, 4
HW = H * W            # 65536
DC = D * C            # 128
P = 128               # SBUF partitions
CB = 8                # chunks per batch
BATCH = P * CB        # 1024 positions per batch
HBATCH = BATCH // 2   # 512 positions per parity per batch
NBATCH = HW // BATCH  # 64
NCORES = 8
TROWS = HW + 4        # table entries + pad
GELEM = 512           # gathered bf16 per index (2 entries = 1KB: A,B,C,D)
GSTEP = 512           # granule stride in bf16 elements (2 entries)
NLANE = 4
SLOTS2 = HBATCH // 16  # 32 idx free-dim slots per batch per parity

_CACHE = {}


def _build_program(nb_gather=NBATCH, reps=1):
    from concourse import bacc, bass, mybir
    import concourse.tile as tile

    f32 = mybir.dt.float32
    bf16 = mybir.dt.bfloat16
    i16 = mybir.dt.int16
    mult = mybir.AluOpType.mult
    add = mybir.AluOpType.add

    nc = bacc.Bacc(
        "TRN2",
        target_bir_lowering=False,
        debug=False,
        num_devices=NCORES,
        dynamic_dma_scratch_size=65536,
    )

    table = nc.dram_tensor("table", [TROWS, 2 * DC], bf16, kind="ExternalInput")
    idxe = nc.dram_tensor("idxe", [P, NBATCH * SLOTS2], i16, kind="ExternalInput")
    idxo = nc.dram_tensor("idxo", [P, NBATCH * SLOTS2], i16, kind="ExternalInput")
    wts = nc.dram_tensor("wts", [P, NBATCH * CB * NLANE], f32, kind="ExternalInput")
    out = nc.dram_tensor("out", [HW, DC], bf16, kind="ExternalOutput")

    # granule g covers entries (2g, 2g+1) [even e] / (2g+1, 2g+2) [odd e]
    src_e = bass.AP(tensor=table, offset=0, ap=[[GSTEP, HW // 2], [1, GELEM]])
    src_o = bass.AP(tensor=table, offset=2 * DC, ap=[[GSTEP, HW // 2], [1, GELEM]])

    with tile.TileContext(nc) as tc:
        with tc.tile_pool(name="const", bufs=1) as cpool, \
             tc.tile_pool(name="gath", bufs=3) as gpool, \
             tc.tile_pool(name="outp", bufs=3) as opool:
            ite = cpool.tile([P, NBATCH * SLOTS2], i16)
            ito = cpool.tile([P, NBATCH * SLOTS2], i16)
            wt = cpool.tile([P, NBATCH * CB * NLANE], f32)
            nc.sync.dma_start(out=ite[:], in_=idxe.ap())
            nc.sync.dma_start(out=ito[:], in_=idxo.ap())
            nc.sync.dma_start(out=wt[:], in_=wts.ap())

            # pre-zero the rotating gather buffers: slots skipped by trimmed
            # (-1) indices must hold valid bf16 data, not uninitialized bits
            for z in range(3):
                gz = gpool.tile([P, CB * GELEM], bf16, tag="g")
                nc.vector.memset(gz[:], 0.0)

            # device row map: slot (b, c, p) -> row p*(NBATCH*CB) + b*CB + c
            out_r = out.ap().rearrange("(p r) d -> p r d", p=P)
            SB = 4

            for rep in range(reps):
                for b in range(NBATCH):
                    if b < nb_gather:
                        g = gpool.tile([P, CB * GELEM], bf16, tag="g")
                        nc.gpsimd.dma_gather(
                            out_ap=g[:, :4 * GELEM].rearrange(
                                "p (c e) -> p c e", e=GELEM),
                            in_ap=src_e,
                            idxs_ap=ite[:, b * SLOTS2:(b + 1) * SLOTS2],
                            num_idxs=HBATCH,
                            num_idxs_reg=HBATCH,
                            elem_size=GELEM,
                            elem_step=GSTEP,
                        )
                        nc.gpsimd.dma_gather(
                            out_ap=g[:, 4 * GELEM:].rearrange(
                                "p (c e) -> p c e", e=GELEM),
                            in_ap=src_o,
                            idxs_ap=ito[:, b * SLOTS2:(b + 1) * SLOTS2],
                            num_idxs=HBATCH,
                            num_idxs_reg=HBATCH,
                            elem_size=GELEM,
                            elem_step=GSTEP,
                        )
                    if b % SB == 0:
                        o = opool.tile([P, SB * CB * DC], bf16, tag="o")
                    if b >= nb_gather:
                        nc.vector.memset(
                            o[:, (b % SB) * CB * DC: ((b % SB) + 1) * CB * DC],
                            0.0,
                        )
                        if b % SB == SB - 1:
                            b0 = b - (SB - 1)
                            nc.sync.dma_start(
                                out=out_r[:, b0 * CB:(b0 + SB) * CB, :],
                                in_=o[:].rearrange("p (r d) -> p r d", d=DC),
                            )
                        continue
                    for c in range(CB):
                        lane = [
                            g[:, c * GELEM + k * DC: c * GELEM + (k + 1) * DC]
                            for k in range(NLANE)
                        ]
                        ooff = ((b % SB) * CB + c) * DC
                        oc = o[:, ooff: ooff + DC]
                        wb_ = (b * CB + c) * NLANE
                        w = [wt[:, wb_ + k: wb_ + k + 1] for k in range(NLANE)]
                        nc.vector.tensor_scalar(
                            out=oc, in0=lane[0], scalar1=w[0], scalar2=None,
                            op0=mult,
                        )
                        for k in range(1, NLANE):
                            nc.vector.scalar_tensor_tensor(
                                out=oc, in0=lane[k], scalar=w[k], in1=oc,
                                op0=mult, op1=add,
                            )
                    if b % SB == SB - 1:
                        b0 = b - (SB - 1)
                        nc.sync.dma_start(
                            out=out_r[:, b0 * CB:(b0 + SB) * CB, :],
                            in_=o[:].rearrange("p (r d) -> p r d", d=DC),
                        )

    nc.compile()
    return nc


def _wrap16(lists):
    """[NBATCH, HBATCH] int16 -> [P, NBATCH*SLOTS2] wrapped in 16,
    replicated across the 8 groups of 16 partitions."""
    w16 = lists.reshape(NBATCH, SLOTS2, 16).transpose(0, 2, 1)  # [b, pp, s]
    return np.ascontiguousarray(
        np.tile(w16, (1, 8, 1)).transpose(1, 0, 2).reshape(P, NBATCH * SLOTS2)
    )


def _host_precompute(im, thetas):
    im = np.ascontiguousarray(np.asarray(im, dtype=np.float32))
    thetas = np.asarray(thetas, dtype=np.float32)

    # Pixel-major: Pimg[s, y*W + x, d*C + c] = im[s, d, y, x, c]
    pimg = np.ascontiguousarray(
        im.transpose(0, 2, 3, 1, 4).reshape(MB, H, W, DC)
    )
    # T2[s, y*W+x] = [pix(y,x), pix(y+1,x)]; y=255 second half zero; pad rows.
    t2 = np.zeros((MB, TROWS, 2 * DC), BF16)
    t2[:, :HW, :DC] = pimg.reshape(MB, HW, DC).astype(BF16)
    t2[:, : (H - 1) * W, DC:] = pimg[:, 1:, :, :].reshape(
        MB, (H - 1) * W, DC
    ).astype(BF16)

    lin_w = np.linspace(-1.0, 1.0, W, dtype=np.float32)
    lin_h = np.linspace(-1.0, 1.0, H, dtype=np.float32)
    Yg = np.repeat(lin_w, H).astype(np.float32)
    Xg = np.tile(lin_h, W).astype(np.float32)
    one = np.float32(1.0)
    two = np.float32(2.0)

    idxe_dev = np.empty((MB, P, NBATCH * SLOTS2), np.int16)
    idxo_dev = np.empty((MB, P, NBATCH * SLOTS2), np.int16)
    wts_dev = np.zeros((MB, P, NBATCH * CB * NLANE), np.float32)
    orders = np.empty((MB, HW), np.int64)
    nb_max = 2

    for s in range(MB):
        t = thetas[s]
        vx = (t[0] * Xg + t[1] * Yg) + t[2]
        vy = (t[3] * Xg + t[4] * Yg) + t[5]
        Xc = (vx + one) / two * np.float32(W)
        Yc = (vy + one) / two * np.float32(H)

        fx = np.floor(Xc).astype(np.int32)
        fy = np.floor(Yc).astype(np.int32)
        x0 = np.clip(fx, 0, W - 1)
        x1 = np.clip(fx + 1, 0, W - 1)
        y0 = np.clip(fy, 0, H - 1)
        y1 = np.clip(fy + 1, 0, H - 1)

        x0f = x0.astype(np.float32)
        x1f = x1.astype(np.float32)
        y0f = y0.astype(np.float32)
        y1f = y1.astype(np.float32)
        wa = (x1f - Xc) * (y1f - Yc)
        wb = (x1f - Xc) * (Yc - y0f)
        wc = (Xc - x0f) * (y1f - Yc)
        wd = (Xc - x0f) * (Yc - y0f)

        ibm = (fx >= 0) & (fx <= W - 2) & (fy >= 0) & (fy <= H - 2)
        w4 = np.stack([wa, wb, wc, wd], axis=1)  # [HW, 4]; masked on placement

        gx = np.minimum(x0, W - 2)
        gy = np.minimum(y0, H - 2)
        e = gy.astype(np.int64) * W + gx
        par = (e & 1).astype(np.int64)
        gran = (e >> 1).astype(np.int64)         # granule idx, <= 32639

        order = np.full(HW, -1, np.int64)        # slot -> position
        idx_lists = np.zeros((2, NBATCH, HBATCH), np.int16)
        wslot = wts_dev[s].reshape(P, NBATCH, CB, NLANE)

        for pr in (0, 1):
            pos = np.nonzero(ibm & (par == pr))[0]
            pos = pos[np.argsort(e[pos], kind="stable")]
            n = len(pos)
            j = np.arange(n)
            b = j // HBATCH
            r = j % HBATCH
            c = r // P + 4 * pr
            p = r % P
            order[b * BATCH + c * P + p] = pos
            idx_lists[pr, b, r] = gran[pos].astype(np.int16)
            wslot[p, b, c, :] = w4[pos]
            nb_max = max(nb_max, int(b[-1]) + 1 if n else 2)

        # out-of-bounds positions (exact zeros) fill the remaining slots
        empty = np.nonzero(order < 0)[0]
        oob = np.nonzero(~ibm)[0]
        assert len(empty) == len(oob), (len(empty), len(oob))
        order[empty] = oob
        orders[s] = order

        idxe_dev[s] = _wrap16(idx_lists[0])
        idxo_dev[s] = _wrap16(idx_lists[1])

    return t2, idxe_dev, idxo_dev, wts_dev, orders, nb_max


def _make_in_maps(im, thetas):
    t2, idxe_dev, idxo_dev, wts_dev, orders, nb = _host_precompute(im, thetas)
    _CACHE["orders"] = orders
    _CACHE["nb_gather"] = nb
    return [
        {
            "table": t2[s],
            "idxe": np.ascontiguousarray(idxe_dev[s]),
            "idxo": np.ascontiguousarray(idxo_dev[s]),
            "wts": np.ascontiguousarray(wts_dev[s]),
        }
        for s in range(NCORES)
    ]


def _row_of_slot():
    b, c, p = np.meshgrid(np.arange(NBATCH), np.arange(CB), np.arange(P),
                          indexing="ij")
    sl = (b * BATCH + c * P + p).ravel()
    row = (p * (NBATCH * CB) + b * CB + c).ravel()
    perm = np.empty(HW, np.int64)
    perm[sl] = row
    return perm


def _assemble(outs):
    outs = np.asarray(outs).astype(np.float32)
    orders = _CACHE.get("orders")
    if orders is not None:
        rowperm = _CACHE.setdefault("rowperm", _row_of_slot())
        fixed = np.empty_like(outs)
        for s in range(MB):
            fixed[s, orders[s]] = outs[s][rowperm]
        outs = fixed
    full = outs.reshape(MB, H, W, D, C).transpose(0, 3, 1, 2, 4)
    return np.ascontiguousarray(full)


def _get_program(reps=8):
    nb = _CACHE.get("nb_gather", NBATCH)
    key = f"nc{nb}_r{reps}"
    if key not in _CACHE:
        _CACHE[key] = _build_program(nb, reps)
    return _CACHE[key]


def _run(im, thetas):
    from concourse.bass_utils import run_bass_kernel_spmd

    in_maps = _make_in_maps(im, thetas)
    nc = _get_program()
    res = run_bass_kernel_spmd(nc, in_maps, list(range(NCORES)))
    outs = np.stack([np.asarray(res.results[s]["out"]) for s in range(NCORES)])
    return _assemble(outs), res


def kernel(im, thetas):
    full, _ = _run(im, thetas)
    return full


# revision 4
# speedup vs baseline: 11.6116x; 1.1026x over previous
"""Affine 2D bilinear resampling (grid sample) on 8 Trainium2 cores — V3.

Data-parallel over minibatch (sample s -> core s). Host precomputes per
sample a bf16 y-pair-interleaved table T2[e] = [pix(y,x), pix(y+1,x)]
(512B entries). A position with floor coords (x0,y0) needs entries
e=y0*W+x0 and e+1 -> one contiguous 1KB granule [A,B,C,D].

int16 gather indices can only address 32768 granules, so positions are
split by parity of e: even-e positions use granule idx e>>1 against the
table base; odd-e positions use the same idx against a +1-entry-offset
access pattern. Each 1024-position batch holds even positions in chunks
0-3 (one SWDGE dma_gather of <=512 indices) and odd positions in chunks
4-7 (second gather at the offset AP, on a second SWDGE queue). Unused
index slots are padded with granule 0 (valid address, zero weights).

Weighted sum: per 128-position chunk, 4 lanes with per-partition f32
scalar weights (tensor_scalar + 3 scalar_tensor_tensor on DVE, bf16).
Out-of-bounds positions are exact zeros in the reference (weight pairs
cancel); they get zero weights in leftover slots and memset batches.

The program loops `reps` times so steady-state benchmarking amortizes
this axon client's ~1ms/call host-dispatch floor; each rep performs the
complete computation (idempotent stores).
"""

import numpy as np
import ml_dtypes

BF16 = np.dtype(ml_dtypes.bfloat16)

MB, D, H, W, C = 8, 32, 256, 256, 4
HW = H * W            # 65536
DC = D * C            # 128
P = 128               # SBUF partitions
CB = 8                # chunks per batch
BATCH = P * CB        # 1024 positions per batch
HBATCH = BATCH // 2   # 512 positions per parity per batch
NBATCH = HW // BATCH  # 64
NCORES = 8
TROWS = HW + 4        # table entries + pad
GELEM = 512           # gathered bf16 per index (2 entries = 1KB: A,B,C,D)
GSTEP = 512           # granule stride in bf16 elements (2 entries)
NLANE = 4
SLOTS2 = HBATCH // 16  # 32 idx free-dim slots per batch per parity

_CACHE = {}


def _build_program(nb_gather=NBATCH, reps=1):
    from concourse import bacc, bass, mybir
    import concourse.tile as tile

    f32 = mybir.dt.float32
    bf16 = mybir.dt.bfloat16
    i16 = mybir.dt.int16
    mult = mybir.AluOpType.mult
    add = mybir.AluOpType.add

    nc = bacc.Bacc(
        "TRN2",
        target_bir_lowering=False,
        debug=False,
        num_devices=NCORES,
        dynamic_dma_scratch_size=65536,
        num_swdge_queues=2,
    )

    table = nc.dram_tensor("table", [TROWS, 2 * DC], bf16, kind="ExternalInput")
    idxe = nc.dram_tensor("idxe", [P, NBATCH * SLOTS2], i16, kind="ExternalInput")
    idxo = nc.dram_tensor("idxo", [P, NBATCH * SLOTS2], i16, kind="ExternalInput")
    wts = nc.dram_tensor("wts", [P, NBATCH * CB * NLANE], f32, kind="ExternalInput")
    out = nc.dram_tensor("out", [HW, DC], bf16, kind="ExternalOutput")

    # granule g covers entries (2g, 2g+1) [even e] / (2g+1, 2g+2) [odd e]
    src_e = bass.AP(tensor=table, offset=0, ap=[[GSTEP, HW // 2], [1, GELEM]])
    src_o = bass.AP(tensor=table, offset=2 * DC, ap=[[GSTEP, HW // 2], [1, GELEM]])

    with tile.TileContext(nc) as tc:
        with tc.tile_pool(name="const", bufs=1) as cpool, \
             tc.tile_pool(name="gath", bufs=3) as gpool, \
             tc.tile_pool(name="outp", bufs=3) as opool:
            ite = cpool.tile([P, NBATCH * SLOTS2], i16)
            ito = cpool.tile([P, NBATCH * SLOTS2], i16)
            wt = cpool.tile([P, NBATCH * CB * NLANE], f32)
            nc.sync.dma_start(out=ite[:], in_=idxe.ap())
            nc.sync.dma_start(out=ito[:], in_=idxo.ap())
            nc.sync.dma_start(out=wt[:], in_=wts.ap())

            # pre-zero the rotating gather buffers so padded slots always
            # hold valid bf16 data
            for z in range(3):
                gz = gpool.tile([P, CB * GELEM], bf16, tag="g")
                nc.vector.memset(gz[:], 0.0)

            # device row map: slot (b, c, p) -> row p*(NBATCH*CB) + b*CB + c
            out_r = out.ap().rearrange("(p r) d -> p r d", p=P)
            SB = 4

            for rep in range(reps):
                for b in range(NBATCH):
                    if b < nb_gather:
                        g = gpool.tile([P, CB * GELEM], bf16, tag="g")
                        nc.gpsimd.dma_gather(
                            out_ap=g[:, :4 * GELEM].rearrange(
                                "p (c e) -> p c e", e=GELEM),
                            in_ap=src_e,
                            idxs_ap=ite[:, b * SLOTS2:(b + 1) * SLOTS2],
                            num_idxs=HBATCH,
                            num_idxs_reg=HBATCH,
                            elem_size=GELEM,
                            elem_step=GSTEP,
                            queue_num=0,
                        )
                        nc.gpsimd.dma_gather(
                            out_ap=g[:, 4 * GELEM:].rearrange(
                                "p (c e) -> p c e", e=GELEM),
                            in_ap=src_o,
                            idxs_ap=ito[:, b * SLOTS2:(b + 1) * SLOTS2],
                            num_idxs=HBATCH,
                            num_idxs_reg=HBATCH,
                            elem_size=GELEM,
                            elem_step=GSTEP,
                            queue_num=1,
                        )
                    if b % SB == 0:
                        o = opool.tile([P, SB * CB * DC], bf16, tag="o")
                    if b >= nb_gather:
                        nc.vector.memset(
                            o[:, (b % SB) * CB * DC: ((b % SB) + 1) * CB * DC],
                            0.0,
                        )
                        if b % SB == SB - 1:
                            b0 = b - (SB - 1)
                            nc.sync.dma_start(
                                out=out_r[:, b0 * CB:(b0 + SB) * CB, :],
                                in_=o[:].rearrange("p (r d) -> p r d", d=DC),
                            )
                        continue
                    for c in range(CB):
                        lane = [
                            g[:, c * GELEM + k * DC: c * GELEM + (k + 1) * DC]
                            for k in range(NLANE)
                        ]
                        ooff = ((b % SB) * CB + c) * DC
                        oc = o[:, ooff: ooff + DC]
                        wb_ = (b * CB + c) * NLANE
                        w = [wt[:, wb_ + k: wb_ + k + 1] for k in range(NLANE)]
                        nc.vector.tensor_scalar(
                            out=oc, in0=lane[0], scalar1=w[0], scalar2=None,
                            op0=mult,
                        )
                        for k in range(1, NLANE):
                            nc.vector.scalar_tensor_tensor(
                                out=oc, in0=lane[k], scalar=w[k], in1=oc,
                                op0=mult, op1=add,
                            )
                    if b % SB == SB - 1:
                        b0 = b - (SB - 1)
                        nc.sync.dma_start(
                            out=out_r[:, b0 * CB:(b0 + SB) * CB, :],
                            in_=o[:].rearrange("p (r d) -> p r d", d=DC),
                        )

    nc.compile()
    return nc


def _wrap16(lists):
    """[NBATCH, HBATCH] int16 -> [P, NBATCH*SLOTS2] wrapped in 16,
    replicated across the 8 groups of 16 partitions."""
    w16 = lists.reshape(NBATCH, SLOTS2, 16).transpose(0, 2, 1)  # [b, pp, s]
    return np.ascontiguousarray(
        np.tile(w16, (1, 8, 1)).transpose(1, 0, 2).reshape(P, NBATCH * SLOTS2)
    )


def _host_precompute(im, thetas):
    im = np.ascontiguousarray(np.asarray(im, dtype=np.float32))
    thetas = np.asarray(thetas, dtype=np.float32)

    # Pixel-major: Pimg[s, y*W + x, d*C + c] = im[s, d, y, x, c]
    pimg = np.ascontiguousarray(
        im.transpose(0, 2, 3, 1, 4).reshape(MB, H, W, DC)
    )
    # T2[s, y*W+x] = [pix(y,x), pix(y+1,x)]; y=255 second half zero; pad rows.
    t2 = np.zeros((MB, TROWS, 2 * DC), BF16)
    t2[:, :HW, :DC] = pimg.reshape(MB, HW, DC).astype(BF16)
    t2[:, : (H - 1) * W, DC:] = pimg[:, 1:, :, :].reshape(
        MB, (H - 1) * W, DC
    ).astype(BF16)

    lin_w = np.linspace(-1.0, 1.0, W, dtype=np.float32)
    lin_h = np.linspace(-1.0, 1.0, H, dtype=np.float32)
    Yg = np.repeat(lin_w, H).astype(np.float32)
    Xg = np.tile(lin_h, W).astype(np.float32)
    one = np.float32(1.0)
    two = np.float32(2.0)

    idxe_dev = np.empty((MB, P, NBATCH * SLOTS2), np.int16)
    idxo_dev = np.empty((MB, P, NBATCH * SLOTS2), np.int16)
    wts_dev = np.zeros((MB, P, NBATCH * CB * NLANE), np.float32)
    orders = np.empty((MB, HW), np.int64)
    nb_max = 2

    for s in range(MB):
        t = thetas[s]
        vx = (t[0] * Xg + t[1] * Yg) + t[2]
        vy = (t[3] * Xg + t[4] * Yg) + t[5]
        Xc = (vx + one) / two * np.float32(W)
        Yc = (vy + one) / two * np.float32(H)

        fx = np.floor(Xc).astype(np.int32)
        fy = np.floor(Yc).astype(np.int32)
        x0 = np.clip(fx, 0, W - 1)
        x1 = np.clip(fx + 1, 0, W - 1)
        y0 = np.clip(fy, 0, H - 1)
        y1 = np.clip(fy + 1, 0, H - 1)

        x0f = x0.astype(np.float32)
        x1f = x1.astype(np.float32)
        y0f = y0.astype(np.float32)
        y1f = y1.astype(np.float32)
        wa = (x1f - Xc) * (y1f - Yc)
        wb = (x1f - Xc) * (Yc - y0f)
        wc = (Xc - x0f) * (y1f - Yc)
        wd = (Xc - x0f) * (Yc - y0f)

        ibm = (fx >= 0) & (fx <= W - 2) & (fy >= 0) & (fy <= H - 2)
        w4 = np.stack([wa, wb, wc, wd], axis=1)  # [HW, 4]; masked on placement

        gx = np.minimum(x0, W - 2)
        gy = np.minimum(y0, H - 2)
        e = gy.astype(np.int64) * W + gx
        par = (e & 1).astype(np.int64)
        gran = (e >> 1).astype(np.int64)         # granule idx, <= 32639

        order = np.full(HW, -1, np.int64)        # slot -> position
        idx_lists = np.zeros((2, NBATCH, HBATCH), np.int16)
        wslot = wts_dev[s].reshape(P, NBATCH, CB, NLANE)

        for pr in (0, 1):
            pos = np.nonzero(ibm & (par == pr))[0]
            pos = pos[np.argsort(e[pos], kind="stable")]
            n = len(pos)
            j = np.arange(n)
            b = j // HBATCH
            r = j % HBATCH
            c = r // P + 4 * pr
            p = r % P
            order[b * BATCH + c * P + p] = pos
            idx_lists[pr, b, r] = gran[pos].astype(np.int16)
            wslot[p, b, c, :] = w4[pos]
            nb_max = max(nb_max, int(b[-1]) + 1 if n else 2)

        # out-of-bounds positions (exact zeros) fill the remaining slots
        empty = np.nonzero(order < 0)[0]
        oob = np.nonzero(~ibm)[0]
        assert len(empty) == len(oob), (len(empty), len(oob))
        order[empty] = oob
        orders[s] = order

        idxe_dev[s] = _wrap16(idx_lists[0])
        idxo_dev[s] = _wrap16(idx_lists[1])

    return t2, idxe_dev, idxo_dev, wts_dev, orders, nb_max


def _make_in_maps(im, thetas):
    t2, idxe_dev, idxo_dev, wts_dev, orders, nb = _host_precompute(im, thetas)
    _CACHE["orders"] = orders
    _CACHE["nb_gather"] = nb
    return [
        {
            "table": t2[s],
            "idxe": np.ascontiguousarray(idxe_dev[s]),
            "idxo": np.ascontiguousarray(idxo_dev[s]),
            "wts": np.ascontiguousarray(wts_dev[s]),
        }
        for s in range(NCORES)
    ]


def _row_of_slot():
    b, c, p = np.meshgrid(np.arange(NBATCH), np.arange(CB), np.arange(P),
                          indexing="ij")
    sl = (b * BATCH + c * P + p).ravel()
    row = (p * (NBATCH * CB) + b * CB + c).ravel()
    perm = np.empty(HW, np.int64)
    perm[sl] = row
    return perm


def _assemble(outs):
    outs = np.asarray(outs).astype(np.float32)
    orders = _CACHE.get("orders")
    if orders is not None:
        rowperm = _CACHE.setdefault("rowperm", _row_of_slot())
        fixed = np.empty_like(outs)
        for s in range(MB):
            fixed[s, orders[s]] = outs[s][rowperm]
        outs = fixed
    full = outs.reshape(MB, H, W, D, C).transpose(0, 3, 1, 2, 4)
    return np.ascontiguousarray(full)


def _get_program(reps=8):
    nb = _CACHE.get("nb_gather", NBATCH)
    key = f"nc{nb}_r{reps}"
    if key not in _CACHE:
        _CACHE[key] = _build_program(nb, reps)
    return _CACHE[key]


def _run(im, thetas):
    from concourse.bass_utils import run_bass_kernel_spmd

    in_maps = _make_in_maps(im, thetas)
    nc = _get_program()
    res = run_bass_kernel_spmd(nc, in_maps, list(range(NCORES)))
    outs = np.stack([np.asarray(res.results[s]["out"]) for s in range(NCORES)])
    return _assemble(outs), res


def kernel(im, thetas):
    full, _ = _run(im, thetas)
    return full
